# revision 10
# baseline (speedup 1.0000x reference)
"""DropToken gather kernel for Trainium2 (8 NeuronCores).

Computes out[b, c, :] = inputs[b, idx[c], :] (the reference's one-hot
matmul is just a row gather). Memory-bound.

Key optimizations over the f32 baseline:
  * bf16 payload: inputs are cast to bf16 host-side and gathered/stored
    as bf16 (rows stay 2 KB >= the 512 B SDMA line-rate floor), halving
    HBM traffic per core to 4 MiB read + 4 MiB write. Output is cast
    back to f32 host-side. Max elementwise rel err ~2^-9 (~2e-3), well
    inside the 2e-2 gate.
  * Wide indirect DMAs: one indirect_dma_start can carry a [128, n]
    offset AP (descriptor i, p-major, gathers row idx[p, t0+j] into out
    chunk i), so the whole 2048-row gather needs a handful of Q7 SWDGE
    emissions instead of 16 (emission was ~1.1-1.4 us per op and paced
    the f32 kernel).

Sharding: core k -> batch b = k//2, cap-half h = k%2. Each core gathers
2048 rows of 2 KB from its batch's [8192, 1024] bf16 slice. Indices are
reshaped host-side to [128, T] so row r = p*T + t lands in partition p,
free-dim slot t; the store to DRAM is then fully contiguous.
"""

import ml_dtypes
import numpy as np

import concourse.bass as bass
import concourse.tile as tile
from concourse import bacc, mybir
from concourse.bass_utils import run_bass_kernel_spmd

B = 4
LENGTH = 8192
EMBED = 1024
CAP = 4096
N_CORES = 8
ROWS_PER_CORE = B * CAP // N_CORES  # 2048
T = ROWS_PER_CORE // 128  # 16 gathered rows per partition

BF16 = True
# Store grouping (in T units): one SBUF tile + one store per group. Early
# groups wide (big store descriptors), tail narrow (short last chain).
GGROUPS = [4, 4, 4, 2, 1, 1]
# WIDE=True issues ONE indirect_dma_start per group with a [128, n] offset
# AP. CoreSim accepts it but HW descriptor ordering differs (wrong results +
# can wedge the device) -- keep False until the HW mapping is understood.
WIDE = False
# Use the purpose-built InstDMAGatherAnt (production MoE ucode): one Q7
# instruction emits descriptors for a whole chunk of rows, removing the
# ~1.4 us/op SWDGE emission that paces the 16-op indirect variant.
USE_DMA_GATHER = True
# dma_gather chunk widths in T units (chunk = n*128 rows -> one gather +
# one store). Wide head keeps SDMA fed cheaply; narrow tail shortens the
# last gather -> last store chain.
DG_CHUNKS = [8, 4, 2, 1, 1]
STRIP_INIT_BARRIER = True

_nc_cache = None
_nc_cache_key = None


def _strip_init_barrier(nc):
    """Remove the Bass-init const memsets and all-engine barrier from the
    entry block. This kernel has no cross-engine deps besides DMA
    semaphores (runtime-zeroed at NEFF load), so engine-boot alignment is
    unnecessary; saves ~3us of startup."""
    blk = nc.m.functions[0].blocks[0]
    blk.instructions = [
        ins
        for ins in blk.instructions
        if not isinstance(
            ins, (mybir.InstMemset, mybir.InstDrain, mybir.InstEventSemaphore)
        )
    ]


def _dt():
    return mybir.dt.bfloat16 if BF16 else mybir.dt.float32


def _np_dt():
    return ml_dtypes.bfloat16 if BF16 else np.float32


def _build_nc_dma_gather():
    """Raw-block variant using InstDMAGatherAnt.

    Index layout (host-prepared, int16): desired[j] = source row for
    gathered slot j, where slot j lands in SBUF dst[j%128, j//128, :].
    The instruction reads index j from idx16[j%16, j//16] (partitions
    0-15, replicated x8 across the 128 partitions for the 8 Q7 cores).
    We want SBUF[p, c] = x[idx_flat[p*T + c]] so the store to DRAM is
    contiguous, i.e. desired = idx_flat.reshape(128, T).T.ravel().
    """
    from contextlib import ExitStack

    assert sum(DG_CHUNKS) == T
    nc = bacc.Bacc(
        "TRN2",
        target_bir_lowering=False,
        debug=False,
        num_devices=N_CORES,
    )
    x = nc.dram_tensor("x", [LENGTH, EMBED], _dt(), kind="ExternalInput").ap()
    idx16 = nc.dram_tensor(
        "idx16", [128, ROWS_PER_CORE // 16], mybir.dt.int16, kind="ExternalInput"
    ).ap()
    out = nc.dram_tensor(
        "out", [128, T * EMBED], _dt(), kind="ExternalOutput"
    ).ap()

    nchunks = len(DG_CHUNKS)
    with ExitStack() as ctx:
        idx_tile = ctx.enter_context(
            nc.sbuf_tensor([128, ROWS_PER_CORE // 16], mybir.dt.int16)
        )
        g = ctx.enter_context(nc.sbuf_tensor([128, T, EMBED], _dt()))
        isem = ctx.enter_context(nc.semaphore("isem"))
        ssem = ctx.enter_context(nc.semaphore("ssem"))
        gsems = [ctx.enter_context(nc.semaphore(f"gsem{i}")) for i in range(nchunks)]
        block = ctx.enter_context(nc.Block())

        @block.scalar
        def _(scalar):
            scalar.dma_start(out=idx_tile[:, :], in_=idx16[:, :]).then_inc(isem, 16)

        @block.gpsimd
        def _(gpsimd):
            gpsimd.wait_ge(isem, 16)
            c0 = 0
            for i, n in enumerate(DG_CHUNKS):
                gpsimd.dma_gather(
                    g[:, c0 : c0 + n, :],
                    x[:, :],
                    idx_tile[:, c0 * 8 : (c0 + n) * 8],
                    n * 128,
                    n * 128,
                    EMBED,
                ).then_inc(gsems[i], 16)
                c0 += n

        @block.sync
        def _(sync):
            c0 = 0
            for i, n in enumerate(DG_CHUNKS):
                sync.wait_ge(gsems[i], 16)
                sync.dma_start(
                    out=out[:, c0 * EMBED : (c0 + n) * EMBED],
                    in_=g[:, c0 : c0 + n, :],
                ).then_inc(ssem, 16)
                c0 += n
            sync.wait_ge(ssem, 16 * nchunks)

    if STRIP_INIT_BARRIER:
        _strip_init_barrier(nc)
    nc.compile()
    return nc


def _build_nc():
    if USE_DMA_GATHER:
        return _build_nc_dma_gather()
    nc = bacc.Bacc(
        "TRN2",
        target_bir_lowering=False,
        debug=False,
        num_devices=N_CORES,
    )
    x = nc.dram_tensor("x", [LENGTH, EMBED], _dt(), kind="ExternalInput").ap()
    idx = nc.dram_tensor("idx", [128, T], mybir.dt.int32, kind="ExternalInput").ap()
    out = nc.dram_tensor(
        "out", [128, T * EMBED], _dt(), kind="ExternalOutput"
    ).ap()

    assert sum(GGROUPS) == T

    with tile.TileContext(nc) as tc:
        with (
            tc.tile_pool(name="idxp", bufs=1) as idxp,
            tc.tile_pool(name="io", bufs=len(GGROUPS)) as io,
        ):
            idx_tile = idxp.tile([128, T], mybir.dt.int32)
            nc.scalar.dma_start(out=idx_tile[:], in_=idx[:, :])
            gmax = max(GGROUPS)
            t0 = 0
            for gw in GGROUPS:
                g = io.tile([128, gmax * EMBED], _dt(), tag="g")
                if WIDE:
                    nc.gpsimd.indirect_dma_start(
                        out=g[:, : gw * EMBED],
                        out_offset=None,
                        in_=x[:, :],
                        in_offset=bass.IndirectOffsetOnAxis(
                            ap=idx_tile[:, t0 : t0 + gw], axis=0
                        ),
                    )
                else:
                    for j in range(gw):
                        t = t0 + j
                        nc.gpsimd.indirect_dma_start(
                            out=g[:, j * EMBED : (j + 1) * EMBED],
                            out_offset=None,
                            in_=x[:, :],
                            in_offset=bass.IndirectOffsetOnAxis(
                                ap=idx_tile[:, t : t + 1], axis=0
                            ),
                        )
                nc.sync.dma_start(
                    out=out[:, t0 * EMBED : (t0 + gw) * EMBED],
                    in_=g[:, : gw * EMBED],
                )
                t0 += gw
    if STRIP_INIT_BARRIER:
        _strip_init_barrier(nc)
    nc.compile()
    return nc


def _get_nc():
    global _nc_cache, _nc_cache_key
    key = (
        BF16,
        tuple(GGROUPS),
        WIDE,
        USE_DMA_GATHER,
        tuple(DG_CHUNKS),
        STRIP_INIT_BARRIER,
    )
    if _nc_cache is None or _nc_cache_key != key:
        _nc_cache = _build_nc()
        _nc_cache_key = key
    return _nc_cache


def _shard_inputs(inputs: np.ndarray, idx: np.ndarray):
    in_maps = []
    half = CAP // 2
    for k in range(N_CORES):
        b, h = divmod(k, 2)
        idx_flat = idx[h * half : (h + 1) * half].astype(np.int32)
        xs = np.ascontiguousarray(inputs[b]).astype(_np_dt())
        if USE_DMA_GATHER:
            # desired[j] = row for gathered slot j (slot j -> dst[j%128, j//128])
            desired = idx_flat.reshape(128, T).T.ravel().astype(np.int16)
            # idx16[p, s] = desired[s*16 + p] for p in 0..15, replicated x8
            wrapped = desired.reshape(ROWS_PER_CORE // 16, 16).T  # [16, R/16]
            idx16 = np.ascontiguousarray(np.tile(wrapped, (8, 1)))
            in_maps.append({"x": xs, "idx16": idx16})
        else:
            shard = np.ascontiguousarray(idx_flat.reshape(128, T))
            in_maps.append({"x": xs, "idx": shard})
    return in_maps


def _run(inputs: np.ndarray, idx: np.ndarray, **run_kwargs):
    nc = _get_nc()
    in_maps = _shard_inputs(inputs, idx)
    res = run_bass_kernel_spmd(nc, in_maps, list(range(N_CORES)), **run_kwargs)
    half = CAP // 2
    out = np.empty((B, CAP, EMBED), np.float32)
    for k in range(N_CORES):
        b, h = divmod(k, 2)
        out[b, h * half : (h + 1) * half] = (
            res.results[k]["out"].reshape(ROWS_PER_CORE, EMBED).astype(np.float32)
        )
    return out, res


def kernel(inputs: np.ndarray, idx: np.ndarray) -> np.ndarray:
    inputs = np.asarray(inputs, dtype=np.float32)
    idx = np.asarray(idx, dtype=np.int32)
    out, _ = _run(inputs, idx)
    return out


# revision 13
# speedup vs baseline: 1.4264x; 1.4264x over previous
"""DropToken gather kernel for Trainium2 (8 NeuronCores).

Computes out[b, c, :] = inputs[b, idx[c], :] (the reference's one-hot
matmul is just a row gather). Memory-bound.

Key optimizations over the f32 baseline:
  * bf16 payload: inputs are cast to bf16 host-side and gathered/stored
    as bf16 (rows stay 2 KB >= the 512 B SDMA line-rate floor), halving
    HBM traffic per core to 4 MiB read + 4 MiB write. Output is cast
    back to f32 host-side. Max elementwise rel err ~2^-9 (~2e-3), well
    inside the 2e-2 gate.
  * Wide indirect DMAs: one indirect_dma_start can carry a [128, n]
    offset AP (descriptor i, p-major, gathers row idx[p, t0+j] into out
    chunk i), so the whole 2048-row gather needs a handful of Q7 SWDGE
    emissions instead of 16 (emission was ~1.1-1.4 us per op and paced
    the f32 kernel).

Sharding: core k -> batch b = k//2, cap-half h = k%2. Each core gathers
2048 rows of 2 KB from its batch's [8192, 1024] bf16 slice. Indices are
reshaped host-side to [128, T] so row r = p*T + t lands in partition p,
free-dim slot t; the store to DRAM is then fully contiguous.
"""

import ml_dtypes
import numpy as np

import concourse.bass as bass
import concourse.tile as tile
from concourse import bacc, mybir
from concourse.bass_utils import run_bass_kernel_spmd

B = 4
LENGTH = 8192
EMBED = 1024
CAP = 4096
N_CORES = 8
ROWS_PER_CORE = B * CAP // N_CORES  # 2048
T = ROWS_PER_CORE // 128  # 16 gathered rows per partition

BF16 = True
# Store grouping (in T units): one SBUF tile + one store per group. Early
# groups wide (big store descriptors), tail narrow (short last chain).
GGROUPS = [4, 4, 4, 2, 1, 1]
# WIDE=True issues ONE indirect_dma_start per group with a [128, n] offset
# AP. CoreSim accepts it but HW descriptor ordering differs (wrong results +
# can wedge the device) -- keep False until the HW mapping is understood.
WIDE = False
# InstDMAGatherAnt variant: one Q7 instruction per chunk, but needs a
# ~10.5 us Q7 library reload before the first gather and emits at ~10
# ns/desc anyway -- measured slower (58.5 us) than the indirect path.
USE_DMA_GATHER = False
DG_CHUNKS = [8, 4, 2, 1, 1]
# Raw-block variant of the indirect path: dedicated semaphore per gather
# op (Tile reuses 8 DMASW lanes, which couples op N's emission to op
# N-8's DMA completion and stretches the emission cadence).
USE_RAW = True
STRIP_INIT_BARRIER = True

_nc_cache = None
_nc_cache_key = None


def _strip_init_barrier(nc):
    """Remove the Bass-init const memsets and all-engine barrier from the
    entry block. This kernel has no cross-engine deps besides DMA
    semaphores (runtime-zeroed at NEFF load), so engine-boot alignment is
    unnecessary; saves ~3us of startup."""
    blk = nc.m.functions[0].blocks[0]
    blk.instructions = [
        ins
        for ins in blk.instructions
        if not isinstance(
            ins, (mybir.InstMemset, mybir.InstDrain, mybir.InstEventSemaphore)
        )
    ]


def _dt():
    return mybir.dt.bfloat16 if BF16 else mybir.dt.float32


def _np_dt():
    return ml_dtypes.bfloat16 if BF16 else np.float32


def _build_nc_dma_gather():
    """Raw-block variant using InstDMAGatherAnt.

    Index layout (host-prepared, int16): desired[j] = source row for
    gathered slot j, where slot j lands in SBUF dst[j%128, j//128, :].
    The instruction reads index j from idx16[j%16, j//16] (partitions
    0-15, replicated x8 across the 128 partitions for the 8 Q7 cores).
    We want SBUF[p, c] = x[idx_flat[p*T + c]] so the store to DRAM is
    contiguous, i.e. desired = idx_flat.reshape(128, T).T.ravel().
    """
    from contextlib import ExitStack

    assert sum(DG_CHUNKS) == T
    nc = bacc.Bacc(
        "TRN2",
        target_bir_lowering=False,
        debug=False,
        num_devices=N_CORES,
    )
    x = nc.dram_tensor("x", [LENGTH, EMBED], _dt(), kind="ExternalInput").ap()
    idx16 = nc.dram_tensor(
        "idx16", [128, ROWS_PER_CORE // 16], mybir.dt.int16, kind="ExternalInput"
    ).ap()
    out = nc.dram_tensor(
        "out", [128, T * EMBED], _dt(), kind="ExternalOutput"
    ).ap()

    nchunks = len(DG_CHUNKS)
    with ExitStack() as ctx:
        idx_tile = ctx.enter_context(
            nc.sbuf_tensor([128, ROWS_PER_CORE // 16], mybir.dt.int16)
        )
        g = ctx.enter_context(nc.sbuf_tensor([128, T, EMBED], _dt()))
        isem = ctx.enter_context(nc.semaphore("isem"))
        ssem = ctx.enter_context(nc.semaphore("ssem"))
        gsems = [ctx.enter_context(nc.semaphore(f"gsem{i}")) for i in range(nchunks)]
        block = ctx.enter_context(nc.Block())

        @block.scalar
        def _(scalar):
            scalar.dma_start(out=idx_tile[:, :], in_=idx16[:, :]).then_inc(isem, 16)

        @block.gpsimd
        def _(gpsimd):
            gpsimd.wait_ge(isem, 16)
            c0 = 0
            for i, n in enumerate(DG_CHUNKS):
                gpsimd.dma_gather(
                    g[:, c0 : c0 + n, :],
                    x[:, :],
                    idx_tile[:, c0 * 8 : (c0 + n) * 8],
                    n * 128,
                    n * 128,
                    EMBED,
                ).then_inc(gsems[i], 16)
                c0 += n

        @block.sync
        def _(sync):
            c0 = 0
            for i, n in enumerate(DG_CHUNKS):
                sync.wait_ge(gsems[i], 16)
                sync.dma_start(
                    out=out[:, c0 * EMBED : (c0 + n) * EMBED],
                    in_=g[:, c0 : c0 + n, :],
                ).then_inc(ssem, 16)
                c0 += n
            sync.wait_ge(ssem, 16 * nchunks)

    if STRIP_INIT_BARRIER:
        _strip_init_barrier(nc)
    nc.compile()
    return nc


def _build_nc_raw():
    """Raw blocks, 16 indirect gathers each with a dedicated semaphore so
    nothing couples Q7 emission of op N to DMA completion of earlier ops.
    Stores taper per GGROUPS; store i waits only on the gathers it covers."""
    from contextlib import ExitStack

    nc = bacc.Bacc(
        "TRN2",
        target_bir_lowering=False,
        debug=False,
        num_devices=N_CORES,
    )
    x = nc.dram_tensor("x", [LENGTH, EMBED], _dt(), kind="ExternalInput").ap()
    idx = nc.dram_tensor("idx", [128, T], mybir.dt.int32, kind="ExternalInput").ap()
    out = nc.dram_tensor(
        "out", [128, T * EMBED], _dt(), kind="ExternalOutput"
    ).ap()

    assert sum(GGROUPS) == T
    with ExitStack() as ctx:
        idx_tile = ctx.enter_context(nc.sbuf_tensor([128, T], mybir.dt.int32))
        g = ctx.enter_context(nc.sbuf_tensor([128, T * EMBED], _dt()))
        isem = ctx.enter_context(nc.semaphore("isem"))
        ssem = ctx.enter_context(nc.semaphore("ssem"))
        gsems = [ctx.enter_context(nc.semaphore(f"gsem{t}")) for t in range(T)]
        block = ctx.enter_context(nc.Block())

        @block.scalar
        def _(scalar):
            scalar.dma_start(out=idx_tile[:, :], in_=idx[:, :]).then_inc(isem, 16)

        @block.gpsimd
        def _(gpsimd):
            gpsimd.wait_ge(isem, 16)
            for t in range(T):
                gpsimd.indirect_dma_start(
                    out=g[:, t * EMBED : (t + 1) * EMBED],
                    out_offset=None,
                    in_=x[:, :],
                    in_offset=bass.IndirectOffsetOnAxis(
                        ap=idx_tile[:, t : t + 1], axis=0
                    ),
                ).then_inc(gsems[t], 16)

        @block.sync
        def _(sync):
            t0 = 0
            for gw in GGROUPS:
                for j in range(gw):
                    sync.wait_ge(gsems[t0 + j], 16)
                sync.dma_start(
                    out=out[:, t0 * EMBED : (t0 + gw) * EMBED],
                    in_=g[:, t0 * EMBED : (t0 + gw) * EMBED],
                ).then_inc(ssem, 16)
                t0 += gw
            sync.wait_ge(ssem, 16 * len(GGROUPS))

    if STRIP_INIT_BARRIER:
        _strip_init_barrier(nc)
    nc.compile()
    return nc


def _build_nc():
    if USE_DMA_GATHER:
        return _build_nc_dma_gather()
    if USE_RAW:
        return _build_nc_raw()
    nc = bacc.Bacc(
        "TRN2",
        target_bir_lowering=False,
        debug=False,
        num_devices=N_CORES,
    )
    x = nc.dram_tensor("x", [LENGTH, EMBED], _dt(), kind="ExternalInput").ap()
    idx = nc.dram_tensor("idx", [128, T], mybir.dt.int32, kind="ExternalInput").ap()
    out = nc.dram_tensor(
        "out", [128, T * EMBED], _dt(), kind="ExternalOutput"
    ).ap()

    assert sum(GGROUPS) == T

    with tile.TileContext(nc) as tc:
        with (
            tc.tile_pool(name="idxp", bufs=1) as idxp,
            tc.tile_pool(name="io", bufs=len(GGROUPS)) as io,
        ):
            idx_tile = idxp.tile([128, T], mybir.dt.int32)
            nc.scalar.dma_start(out=idx_tile[:], in_=idx[:, :])
            gmax = max(GGROUPS)
            t0 = 0
            for gw in GGROUPS:
                g = io.tile([128, gmax * EMBED], _dt(), tag="g")
                if WIDE:
                    nc.gpsimd.indirect_dma_start(
                        out=g[:, : gw * EMBED],
                        out_offset=None,
                        in_=x[:, :],
                        in_offset=bass.IndirectOffsetOnAxis(
                            ap=idx_tile[:, t0 : t0 + gw], axis=0
                        ),
                    )
                else:
                    for j in range(gw):
                        t = t0 + j
                        nc.gpsimd.indirect_dma_start(
                            out=g[:, j * EMBED : (j + 1) * EMBED],
                            out_offset=None,
                            in_=x[:, :],
                            in_offset=bass.IndirectOffsetOnAxis(
                                ap=idx_tile[:, t : t + 1], axis=0
                            ),
                        )
                nc.sync.dma_start(
                    out=out[:, t0 * EMBED : (t0 + gw) * EMBED],
                    in_=g[:, : gw * EMBED],
                )
                t0 += gw
    if STRIP_INIT_BARRIER:
        _strip_init_barrier(nc)
    nc.compile()
    return nc


def _get_nc():
    global _nc_cache, _nc_cache_key
    key = (
        BF16,
        tuple(GGROUPS),
        WIDE,
        USE_DMA_GATHER,
        tuple(DG_CHUNKS),
        USE_RAW,
        STRIP_INIT_BARRIER,
    )
    if _nc_cache is None or _nc_cache_key != key:
        _nc_cache = _build_nc()
        _nc_cache_key = key
    return _nc_cache


def _shard_inputs(inputs: np.ndarray, idx: np.ndarray):
    in_maps = []
    half = CAP // 2
    for k in range(N_CORES):
        b, h = divmod(k, 2)
        idx_flat = idx[h * half : (h + 1) * half].astype(np.int32)
        xs = np.ascontiguousarray(inputs[b]).astype(_np_dt())
        if USE_DMA_GATHER:
            # desired[j] = row for gathered slot j (slot j -> dst[j%128, j//128])
            desired = idx_flat.reshape(128, T).T.ravel().astype(np.int16)
            # idx16[p, s] = desired[s*16 + p] for p in 0..15, replicated x8
            wrapped = desired.reshape(ROWS_PER_CORE // 16, 16).T  # [16, R/16]
            idx16 = np.ascontiguousarray(np.tile(wrapped, (8, 1)))
            in_maps.append({"x": xs, "idx16": idx16})
        else:
            shard = np.ascontiguousarray(idx_flat.reshape(128, T))
            in_maps.append({"x": xs, "idx": shard})
    return in_maps


def _run(inputs: np.ndarray, idx: np.ndarray, **run_kwargs):
    nc = _get_nc()
    in_maps = _shard_inputs(inputs, idx)
    res = run_bass_kernel_spmd(nc, in_maps, list(range(N_CORES)), **run_kwargs)
    half = CAP // 2
    out = np.empty((B, CAP, EMBED), np.float32)
    for k in range(N_CORES):
        b, h = divmod(k, 2)
        out[b, h * half : (h + 1) * half] = (
            res.results[k]["out"].reshape(ROWS_PER_CORE, EMBED).astype(np.float32)
        )
    return out, res


def kernel(inputs: np.ndarray, idx: np.ndarray) -> np.ndarray:
    inputs = np.asarray(inputs, dtype=np.float32)
    idx = np.asarray(idx, dtype=np.int32)
    out, _ = _run(inputs, idx)
    return out


# revision 19
# speedup vs baseline: 1.7918x; 1.2561x over previous
"""DropToken gather kernel for Trainium2 (8 NeuronCores).

Computes out[b, c, :] = inputs[b, idx[c], :] (the reference's one-hot
matmul is just a row gather). Memory-bound.

Key optimizations over the f32 baseline:
  * bf16 payload: inputs are cast to bf16 host-side and gathered/stored
    as bf16 (rows stay 2 KB >= the 512 B SDMA line-rate floor), halving
    HBM traffic per core to 4 MiB read + 4 MiB write. Output is cast
    back to f32 host-side. Max elementwise rel err ~2^-9 (~2e-3), well
    inside the 2e-2 gate.
  * Wide indirect DMAs: one indirect_dma_start can carry a [128, n]
    offset AP (descriptor i, p-major, gathers row idx[p, t0+j] into out
    chunk i), so the whole 2048-row gather needs a handful of Q7 SWDGE
    emissions instead of 16 (emission was ~1.1-1.4 us per op and paced
    the f32 kernel).

Sharding: core k -> batch b = k//2, cap-half h = k%2. Each core gathers
2048 rows of 2 KB from its batch's [8192, 1024] bf16 slice. Indices are
reshaped host-side to [128, T] so row r = p*T + t lands in partition p,
free-dim slot t; the store to DRAM is then fully contiguous.
"""

import ml_dtypes
import numpy as np

import concourse.bass as bass
import concourse.tile as tile
from concourse import bacc, mybir
from concourse.bass_utils import run_bass_kernel_spmd

B = 4
LENGTH = 8192
EMBED = 1024
CAP = 4096
N_CORES = 8
ROWS_PER_CORE = B * CAP // N_CORES  # 2048
T = ROWS_PER_CORE // 128  # 16 gathered rows per partition

BF16 = True
# Store grouping (in T units): one SBUF tile + one store per group. Early
# groups wide (big store descriptors), tail narrow (short last chain).
GGROUPS = [4, 4, 4, 2, 1, 1]
# WIDE=True issues ONE indirect_dma_start per group with a [128, n] offset
# AP. CoreSim accepts it but HW descriptor ordering differs (wrong results +
# can wedge the device) -- keep False until the HW mapping is understood.
WIDE = False
# InstDMAGatherAnt variant: one Q7 instruction per chunk, but needs a
# ~10.5 us Q7 library reload before the first gather and emits at ~10
# ns/desc anyway -- measured slower (58.5 us) than the indirect path.
USE_DMA_GATHER = False
DG_CHUNKS = [8, 4, 2, 1, 1]
# Raw-block variant of the indirect path: dedicated semaphore per gather
# op (Tile reuses 8 DMASW lanes, which couples op N's emission to op
# N-8's DMA completion and stretches the emission cadence).
USE_RAW = True
# Batch-interleaved gather: upload x as [LENGTH, IL*EMBED] with IL batches
# concatenated per row (all batches share idx), so one descriptor fetches
# IL rows at once. IL=4: 512 descs/core in 4 ops (Q7 emission ~5.6 us,
# fully hidden) and 8 KB random reads (vs 2 KB) for better HBM efficiency.
# HW semantics probe-validated: offsets [128,1], per-desc length = dest
# partition-row bytes.
INTERLEAVE = 4
# gather op widths in cap-positions-per-partition units (sum = CAP/N_CORES/128
# = 4 for IL=4); one [128,1]-offset op + one store per unit of 1.
IL_OPS = 4
STRIP_INIT_BARRIER = True

_nc_cache = None
_nc_cache_key = None


def _strip_init_barrier(nc):
    """Remove the Bass-init const memsets and all-engine barrier from the
    entry block. This kernel has no cross-engine deps besides DMA
    semaphores (runtime-zeroed at NEFF load), so engine-boot alignment is
    unnecessary; saves ~3us of startup."""
    blk = nc.m.functions[0].blocks[0]
    blk.instructions = [
        ins
        for ins in blk.instructions
        if not isinstance(
            ins, (mybir.InstMemset, mybir.InstDrain, mybir.InstEventSemaphore)
        )
    ]


def _dt():
    return mybir.dt.bfloat16 if BF16 else mybir.dt.float32


def _np_dt():
    return ml_dtypes.bfloat16 if BF16 else np.float32


def _build_nc_dma_gather():
    """Raw-block variant using InstDMAGatherAnt.

    Index layout (host-prepared, int16): desired[j] = source row for
    gathered slot j, where slot j lands in SBUF dst[j%128, j//128, :].
    The instruction reads index j from idx16[j%16, j//16] (partitions
    0-15, replicated x8 across the 128 partitions for the 8 Q7 cores).
    We want SBUF[p, c] = x[idx_flat[p*T + c]] so the store to DRAM is
    contiguous, i.e. desired = idx_flat.reshape(128, T).T.ravel().
    """
    from contextlib import ExitStack

    assert sum(DG_CHUNKS) == T
    nc = bacc.Bacc(
        "TRN2",
        target_bir_lowering=False,
        debug=False,
        num_devices=N_CORES,
    )
    x = nc.dram_tensor("x", [LENGTH, EMBED], _dt(), kind="ExternalInput").ap()
    idx16 = nc.dram_tensor(
        "idx16", [128, ROWS_PER_CORE // 16], mybir.dt.int16, kind="ExternalInput"
    ).ap()
    out = nc.dram_tensor(
        "out", [128, T * EMBED], _dt(), kind="ExternalOutput"
    ).ap()

    nchunks = len(DG_CHUNKS)
    with ExitStack() as ctx:
        idx_tile = ctx.enter_context(
            nc.sbuf_tensor([128, ROWS_PER_CORE // 16], mybir.dt.int16)
        )
        g = ctx.enter_context(nc.sbuf_tensor([128, T, EMBED], _dt()))
        isem = ctx.enter_context(nc.semaphore("isem"))
        ssem = ctx.enter_context(nc.semaphore("ssem"))
        gsems = [ctx.enter_context(nc.semaphore(f"gsem{i}")) for i in range(nchunks)]
        block = ctx.enter_context(nc.Block())

        @block.scalar
        def _(scalar):
            scalar.dma_start(out=idx_tile[:, :], in_=idx16[:, :]).then_inc(isem, 16)

        @block.gpsimd
        def _(gpsimd):
            gpsimd.wait_ge(isem, 16)
            c0 = 0
            for i, n in enumerate(DG_CHUNKS):
                gpsimd.dma_gather(
                    g[:, c0 : c0 + n, :],
                    x[:, :],
                    idx_tile[:, c0 * 8 : (c0 + n) * 8],
                    n * 128,
                    n * 128,
                    EMBED,
                ).then_inc(gsems[i], 16)
                c0 += n

        @block.sync
        def _(sync):
            c0 = 0
            for i, n in enumerate(DG_CHUNKS):
                sync.wait_ge(gsems[i], 16)
                sync.dma_start(
                    out=out[:, c0 * EMBED : (c0 + n) * EMBED],
                    in_=g[:, c0 : c0 + n, :],
                ).then_inc(ssem, 16)
                c0 += n
            sync.wait_ge(ssem, 16 * nchunks)

    if STRIP_INIT_BARRIER:
        _strip_init_barrier(nc)
    nc.compile()
    return nc


def _build_nc_il():
    """Batch-interleaved gather: x is [LENGTH, IL*EMBED] (IL batches per
    row), each core covers CAP/N_CORES cap positions with one 8 KB
    descriptor per position. IL_OPS ops of [128,1] offsets; store per op."""
    from contextlib import ExitStack

    ilw = INTERLEAVE * EMBED  # elems per interleaved row
    nc = bacc.Bacc(
        "TRN2",
        target_bir_lowering=False,
        debug=False,
        num_devices=N_CORES,
    )
    x = nc.dram_tensor("x", [LENGTH, ilw], _dt(), kind="ExternalInput").ap()
    idx = nc.dram_tensor(
        "idx", [128, IL_OPS], mybir.dt.int32, kind="ExternalInput"
    ).ap()
    out = nc.dram_tensor(
        "out", [128, IL_OPS * ilw], _dt(), kind="ExternalOutput"
    ).ap()

    with ExitStack() as ctx:
        idx_tile = ctx.enter_context(nc.sbuf_tensor([128, IL_OPS], mybir.dt.int32))
        g = ctx.enter_context(nc.sbuf_tensor([128, IL_OPS * ilw], _dt()))
        isem = ctx.enter_context(nc.semaphore("isem"))
        ssem = ctx.enter_context(nc.semaphore("ssem"))
        gsems = [ctx.enter_context(nc.semaphore(f"gsem{o}")) for o in range(IL_OPS)]
        block = ctx.enter_context(nc.Block())

        @block.scalar
        def _(scalar):
            scalar.dma_start(out=idx_tile[:, :], in_=idx[:, :]).then_inc(isem, 16)

        @block.gpsimd
        def _(gpsimd):
            gpsimd.wait_ge(isem, 16)
            for o in range(IL_OPS):
                gpsimd.indirect_dma_start(
                    out=g[:, o * ilw : (o + 1) * ilw],
                    out_offset=None,
                    in_=x[:, :],
                    in_offset=bass.IndirectOffsetOnAxis(
                        ap=idx_tile[:, o : o + 1], axis=0
                    ),
                ).then_inc(gsems[o], 16)

        @block.sync
        def _(sync):
            for o in range(IL_OPS):
                sync.wait_ge(gsems[o], 16)
                sync.dma_start(
                    out=out[:, o * ilw : (o + 1) * ilw],
                    in_=g[:, o * ilw : (o + 1) * ilw],
                ).then_inc(ssem, 16)
            sync.wait_ge(ssem, 16 * IL_OPS)

    if STRIP_INIT_BARRIER:
        _strip_init_barrier(nc)
    nc.compile()
    return nc


def _build_nc_raw():
    """Raw blocks, 16 indirect gathers each with a dedicated semaphore so
    nothing couples Q7 emission of op N to DMA completion of earlier ops.
    Stores taper per GGROUPS; store i waits only on the gathers it covers."""
    from contextlib import ExitStack

    nc = bacc.Bacc(
        "TRN2",
        target_bir_lowering=False,
        debug=False,
        num_devices=N_CORES,
    )
    x = nc.dram_tensor("x", [LENGTH, EMBED], _dt(), kind="ExternalInput").ap()
    idx = nc.dram_tensor("idx", [128, T], mybir.dt.int32, kind="ExternalInput").ap()
    out = nc.dram_tensor(
        "out", [128, T * EMBED], _dt(), kind="ExternalOutput"
    ).ap()

    assert sum(GGROUPS) == T
    with ExitStack() as ctx:
        idx_tile = ctx.enter_context(nc.sbuf_tensor([128, T], mybir.dt.int32))
        g = ctx.enter_context(nc.sbuf_tensor([128, T * EMBED], _dt()))
        isem = ctx.enter_context(nc.semaphore("isem"))
        ssem = ctx.enter_context(nc.semaphore("ssem"))
        gsems = [ctx.enter_context(nc.semaphore(f"gsem{t}")) for t in range(T)]
        block = ctx.enter_context(nc.Block())

        @block.scalar
        def _(scalar):
            scalar.dma_start(out=idx_tile[:, :], in_=idx[:, :]).then_inc(isem, 16)

        @block.gpsimd
        def _(gpsimd):
            gpsimd.wait_ge(isem, 16)
            for t in range(T):
                gpsimd.indirect_dma_start(
                    out=g[:, t * EMBED : (t + 1) * EMBED],
                    out_offset=None,
                    in_=x[:, :],
                    in_offset=bass.IndirectOffsetOnAxis(
                        ap=idx_tile[:, t : t + 1], axis=0
                    ),
                ).then_inc(gsems[t], 16)

        @block.sync
        def _(sync):
            t0 = 0
            for gw in GGROUPS:
                for j in range(gw):
                    sync.wait_ge(gsems[t0 + j], 16)
                sync.dma_start(
                    out=out[:, t0 * EMBED : (t0 + gw) * EMBED],
                    in_=g[:, t0 * EMBED : (t0 + gw) * EMBED],
                ).then_inc(ssem, 16)
                t0 += gw
            sync.wait_ge(ssem, 16 * len(GGROUPS))

    if STRIP_INIT_BARRIER:
        _strip_init_barrier(nc)
    nc.compile()
    return nc


def _build_nc():
    if INTERLEAVE > 1:
        return _build_nc_il()
    if USE_DMA_GATHER:
        return _build_nc_dma_gather()
    if USE_RAW:
        return _build_nc_raw()
    nc = bacc.Bacc(
        "TRN2",
        target_bir_lowering=False,
        debug=False,
        num_devices=N_CORES,
    )
    x = nc.dram_tensor("x", [LENGTH, EMBED], _dt(), kind="ExternalInput").ap()
    idx = nc.dram_tensor("idx", [128, T], mybir.dt.int32, kind="ExternalInput").ap()
    out = nc.dram_tensor(
        "out", [128, T * EMBED], _dt(), kind="ExternalOutput"
    ).ap()

    assert sum(GGROUPS) == T

    with tile.TileContext(nc) as tc:
        with (
            tc.tile_pool(name="idxp", bufs=1) as idxp,
            tc.tile_pool(name="io", bufs=len(GGROUPS)) as io,
        ):
            idx_tile = idxp.tile([128, T], mybir.dt.int32)
            nc.scalar.dma_start(out=idx_tile[:], in_=idx[:, :])
            gmax = max(GGROUPS)
            t0 = 0
            for gw in GGROUPS:
                g = io.tile([128, gmax * EMBED], _dt(), tag="g")
                if WIDE:
                    nc.gpsimd.indirect_dma_start(
                        out=g[:, : gw * EMBED],
                        out_offset=None,
                        in_=x[:, :],
                        in_offset=bass.IndirectOffsetOnAxis(
                            ap=idx_tile[:, t0 : t0 + gw], axis=0
                        ),
                    )
                else:
                    for j in range(gw):
                        t = t0 + j
                        nc.gpsimd.indirect_dma_start(
                            out=g[:, j * EMBED : (j + 1) * EMBED],
                            out_offset=None,
                            in_=x[:, :],
                            in_offset=bass.IndirectOffsetOnAxis(
                                ap=idx_tile[:, t : t + 1], axis=0
                            ),
                        )
                nc.sync.dma_start(
                    out=out[:, t0 * EMBED : (t0 + gw) * EMBED],
                    in_=g[:, : gw * EMBED],
                )
                t0 += gw
    if STRIP_INIT_BARRIER:
        _strip_init_barrier(nc)
    nc.compile()
    return nc


def _get_nc():
    global _nc_cache, _nc_cache_key
    key = (
        BF16,
        tuple(GGROUPS),
        WIDE,
        USE_DMA_GATHER,
        tuple(DG_CHUNKS),
        USE_RAW,
        INTERLEAVE,
        IL_OPS,
        STRIP_INIT_BARRIER,
    )
    if _nc_cache is None or _nc_cache_key != key:
        _nc_cache = _build_nc()
        _nc_cache_key = key
    return _nc_cache


def _shard_inputs(inputs: np.ndarray, idx: np.ndarray):
    in_maps = []
    if INTERLEAVE > 1:
        assert INTERLEAVE == B
        # x_il[l] = [inputs[0,l,:] | inputs[1,l,:] | ...] -- shared by cores
        x_il = np.ascontiguousarray(
            inputs.transpose(1, 0, 2).reshape(LENGTH, B * EMBED).astype(_np_dt())
        )
        per = CAP // N_CORES  # cap positions per core (512)
        for k in range(N_CORES):
            # position k*per + o*128 + p  ->  op o, partition p
            idx_t = np.ascontiguousarray(
                idx[k * per : (k + 1) * per].reshape(IL_OPS, 128).T.astype(np.int32)
            )
            in_maps.append({"x": x_il, "idx": idx_t})
        return in_maps
    half = CAP // 2
    for k in range(N_CORES):
        b, h = divmod(k, 2)
        idx_flat = idx[h * half : (h + 1) * half].astype(np.int32)
        xs = np.ascontiguousarray(inputs[b]).astype(_np_dt())
        if USE_DMA_GATHER:
            # desired[j] = row for gathered slot j (slot j -> dst[j%128, j//128])
            desired = idx_flat.reshape(128, T).T.ravel().astype(np.int16)
            # idx16[p, s] = desired[s*16 + p] for p in 0..15, replicated x8
            wrapped = desired.reshape(ROWS_PER_CORE // 16, 16).T  # [16, R/16]
            idx16 = np.ascontiguousarray(np.tile(wrapped, (8, 1)))
            in_maps.append({"x": xs, "idx16": idx16})
        else:
            shard = np.ascontiguousarray(idx_flat.reshape(128, T))
            in_maps.append({"x": xs, "idx": shard})
    return in_maps


def _run(inputs: np.ndarray, idx: np.ndarray, **run_kwargs):
    nc = _get_nc()
    in_maps = _shard_inputs(inputs, idx)
    res = run_bass_kernel_spmd(nc, in_maps, list(range(N_CORES)), **run_kwargs)
    out = np.empty((B, CAP, EMBED), np.float32)
    if INTERLEAVE > 1:
        per = CAP // N_CORES
        for k in range(N_CORES):
            arr = (
                res.results[k]["out"]
                .reshape(128, IL_OPS, B, EMBED)
                .astype(np.float32)
            )
            # [p, o, b, e] -> [b, o*128+p, e]
            out[:, k * per : (k + 1) * per] = arr.transpose(2, 1, 0, 3).reshape(
                B, per, EMBED
            )
        return out, res
    half = CAP // 2
    for k in range(N_CORES):
        b, h = divmod(k, 2)
        out[b, h * half : (h + 1) * half] = (
            res.results[k]["out"].reshape(ROWS_PER_CORE, EMBED).astype(np.float32)
        )
    return out, res


def kernel(inputs: np.ndarray, idx: np.ndarray) -> np.ndarray:
    inputs = np.asarray(inputs, dtype=np.float32)
    idx = np.asarray(idx, dtype=np.int32)
    out, _ = _run(inputs, idx)
    return out
